# revision 30
# baseline (speedup 1.0000x reference)
"""Trainium2 Bass kernel for nn_Cluster_loss (discriminative/cluster loss).

Strategy (8 NeuronCores, full inputs in / full outputs back):
  - Host sorts each image's pixels by ground-truth cluster label into 16
    contiguous segments, split over 4 partition row-groups (j) of 32
    channels each -> feature layout (128, F) bf16 per core.
  - One full image per core (2 cores per image). Each core computes the
    image's cluster sums (mu) redundantly -- this avoids any mid-kernel
    cross-core collective.  Each core runs the per-pixel distance pass
    on the half of the pixels it "owns" (placed first in each segment).
  - Pass 1 (mu): per-segment DVE tensor_scalar with accum_out (free-axis
    accumulate), then a tiny fp32 matmul folds the 4 j-groups.
  - Pass 2: diff = feat - mu[cluster] via per-partition scalar subtract,
    square, then per-128-column-chunk matmul with the chunk as the
    stationary operand and a block-diagonal selector as moving operand:
    out[f, j] = sum_c diffsq[(j,c), f]  (pixels land on partitions).
    ACT Sqrt moves distances to SBUF; per-cluster ones-matmul column
    sums feed per-(chunk,j) partial sums shipped to the host.
  - Host assembles the scalar losses (float64) from mu + per-cluster
    distance sums, correcting analytically for zero pad pixels.
"""

import math
import os
import sys

import numpy as np

try:
    import ml_dtypes
except ImportError:  # pragma: no cover
    ml_dtypes = None

_TRN_REPO = "/opt/trn_rl_repo"
if _TRN_REPO not in sys.path:
    sys.path.insert(0, _TRN_REPO)

N, C, H, W = 4, 32, 256, 512
P = H * W          # pixels per image
K = 16             # clusters
NJ = 4             # partition row-groups (4 x 32 channels = 128 partitions)
NCORES = 8
DELTA_V = 0.2
DELTA_D = 0.2
ALPHA, BETA, GAMMA = 1.0, 1.0, 0.001

_BF16 = np.dtype(ml_dtypes.bfloat16) if ml_dtypes is not None else None

# Engine assignment knobs (load balance).
# pass 1 accumulation: which engine reduces each cluster segment.
# Interleaved with DMA arrival order so both engines chase the stream;
# ACT is faster per segment (1.95us vs 2.67us) so it takes more + the tail.
_P1_ACT_SET = frozenset({1, 3, 5, 7, 9, 11})
# last-arriving segments: accumulate the two halves on DVE+ACT in parallel
_P1_SPLIT = (12, 13, 14, 15)
# pass 2: ACT does fused Square(feat-mu) for these; DVE two-op for rest
_ACT_SQUARE_SET = frozenset({0, 2, 4, 6, 8, 10, 12})
# of the DVE-route clusters, these get their square on GpSimd (idle engine)
_POOL_SQUARE_SET = frozenset({1, 9})


def _mu_perm():
    """Column order of mu_out (must match _build_program's perm)."""
    dve_fulls = [k for k in range(K)
                 if k not in _P1_ACT_SET and k not in _P1_SPLIT]
    return dve_fulls + sorted(_P1_ACT_SET) + list(_P1_SPLIT)


def _roundup(x, m):
    return (x + m - 1) // m * m


def _build_layout(gt_flat):
    """Compute global (shared across cores) segment geometry from labels.

    Returns dict with per-cluster H (own-half region width) and Q = 2H
    (full segment width), plus offsets.  All multiples of 8 columns.
    """
    # idx[n][k] = pixel indices of cluster k in image n
    idx = [[np.nonzero(gt_flat[n] == k)[0] for k in range(K)] for n in range(N)]
    Hk = np.zeros(K, dtype=np.int64)
    for k in range(K):
        m = 0
        for n in range(N):
            L = len(idx[n][k])
            for j in range(NJ):
                Lj = L // NJ + (1 if j < L % NJ else 0)
                m = max(m, (Lj + 1) // 2)
        Hk[k] = _roundup(max(m, 8), 8)
    Qk = 2 * Hk
    offs = np.zeros(K + 1, dtype=np.int64)
    np.cumsum(Qk, out=offs[1:])
    return idx, Hk, Qk, offs, int(offs[-1])


def _build_core_inputs(feat, idx, Hk, Qk, offs, F):
    """Per-core (128, F) bf16 feature buffers + pad bookkeeping.

    Core c handles image n = c // 2, half h = c % 2.  For each (j, k)
    the segment columns are [off, off+Hk): own pixels then zero pads;
    [off+Hk, off+Qk): partner pixels then zero pads.
    Returns (bufs, npad[core, k] total own-region pad count per cluster).
    """
    bufs = []
    npad = np.zeros((NCORES, K), dtype=np.int64)
    feat_bf = [np.ascontiguousarray(feat[n].reshape(C, P)).astype(_BF16)
               for n in range(N)]
    for core in range(NCORES):
        n, h = core // 2, core % 2
        colsrc = np.zeros((NJ, F), dtype=np.int64)
        valid = np.zeros((NJ, F), dtype=bool)
        for k in range(K):
            ids = idx[n][k]
            L = len(ids)
            pos = 0
            for j in range(NJ):
                Lj = L // NJ + (1 if j < L % NJ else 0)
                part = ids[pos:pos + Lj]
                pos += Lj
                a_sz = (Lj + 1) // 2
                own = part[:a_sz] if h == 0 else part[a_sz:]
                other = part[a_sz:] if h == 0 else part[:a_sz]
                o = offs[k]
                colsrc[j, o:o + len(own)] = own
                valid[j, o:o + len(own)] = True
                npad[core, k] += Hk[k] - len(own)
                o2 = offs[k] + Hk[k]
                colsrc[j, o2:o2 + len(other)] = other
                valid[j, o2:o2 + len(other)] = True
        buf = np.zeros((128, F), dtype=_BF16)
        fb = feat_bf[n]
        for j in range(NJ):
            block = fb[:, colsrc[j]]          # (32, F) gather
            block[:, ~valid[j]] = 0
            buf[j * 32:(j + 1) * 32, :] = block
        bufs.append(buf)
    return bufs, npad


def _build_program(Hk, Qk, offs, F):
    import concourse.bass as bass
    import concourse.tile as tile
    from concourse import bacc, mybir
    from contextlib import ExitStack

    bf = mybir.dt.bfloat16
    f32 = mybir.dt.float32
    nc = bacc.Bacc("TRN2", target_bir_lowering=False, debug=False)

    nch_k = [int(math.ceil(Hk[k] / 128)) for k in range(K)]
    maxnch = max(nch_k)

    feat_in = nc.declare_dram_parameter("feat", [128, F], bf, isOutput=False)
    sel4_in = nc.declare_dram_parameter("sel4", [128, 4], bf, isOutput=False)
    zeros_in = nc.declare_dram_parameter("zeros4", [128, 4 * maxnch], bf,
                                         isOutput=False)
    fold_in = nc.declare_dram_parameter("fold", [128, 128], f32, isOutput=False)
    ones_in = nc.declare_dram_parameter("ones1", [128, 1], bf, isOutput=False)
    mu_out = nc.declare_dram_parameter("mu_out", [128, K], f32, isOutput=True)
    scols = 4 * sum(nch_k)
    s_out = nc.declare_dram_parameter("s_out", [1, scols], f32, isOutput=True)

    maxQ = int(max(Qk))
    maxH = int(max(Hk))

    with tile.TileContext(nc) as tc, ExitStack() as ctx:
        consts = ctx.enter_context(tc.tile_pool(name="consts", bufs=1))
        featp = ctx.enter_context(tc.tile_pool(name="featp", bufs=1))
        scr = ctx.enter_context(tc.tile_pool(name="scr", bufs=2))
        diffp = ctx.enter_context(tc.tile_pool(name="diffp", bufs=4))
        dsqp = ctx.enter_context(tc.tile_pool(name="dsqp", bufs=4))
        dtp = ctx.enter_context(tc.tile_pool(name="dtp", bufs=4))
        smallp = ctx.enter_context(tc.tile_pool(name="smallp", bufs=1))
        psum = ctx.enter_context(tc.tile_pool(name="psum", bufs=5, space="PSUM"))
        psum1 = ctx.enter_context(tc.tile_pool(name="psum1", bufs=1, space="PSUM"))

        # warm the ACT Sqrt table before anything else needs ACT
        warm = smallp.tile([128, 8], bf)
        nc.vector.memset(warm[:], 0.0)
        nc.scalar.activation(out=warm[:], in_=warm[:],
                             func=mybir.ActivationFunctionType.Sqrt)

        sel4_t = consts.tile([128, 4], bf)
        nc.scalar.dma_start(out=sel4_t[:], in_=sel4_in[:])
        zeros_t = consts.tile([128, 4 * maxnch], bf)
        nc.scalar.dma_start(out=zeros_t[:], in_=zeros_in[:])
        fold_t = consts.tile([128, 128], f32)
        nc.scalar.dma_start(out=fold_t[:], in_=fold_in[:])
        ones_b = consts.tile([128, 1], bf)
        nc.scalar.dma_start(out=ones_b[:], in_=ones_in[:])

        # feature loads: split the issue across two otherwise-idle
        # sequencers (SP + GpSimd) -- one sequencer can only start a DMA
        # every ~1.2us, which otherwise rate-limits the whole load phase.
        feat_k = []
        for k in range(K):
            o, q = int(offs[k]), int(Qk[k])
            ft = featp.tile([128, q], bf, tag=f"feat{k}")
            eng = nc.sync if k % 2 == 0 else nc.gpsimd
            if k in _P1_SPLIT:
                half = (q // 2) // 8 * 8
                eng.dma_start(out=ft[:, 0:half], in_=feat_in[:, o:o + half])
                eng.dma_start(out=ft[:, half:q], in_=feat_in[:, o + half:o + q])
            else:
                eng.dma_start(out=ft[:], in_=feat_in[:, o:o + q])
            feat_k.append(ft)

        # ---- pass 1: per-segment accumulate ----
        # Separate per-engine accumulator tiles: a shared tile would
        # serialize DVE/ACT accums on whole-tile WAW tracking.
        nsplit = len(_P1_SPLIT)
        dve_fulls = [k for k in range(K)
                     if k not in _P1_ACT_SET and k not in _P1_SPLIT]
        act_fulls = sorted(_P1_ACT_SET)
        # mu_rep column order: [dve_fulls | act_fulls | splits]
        perm = dve_fulls + act_fulls + list(_P1_SPLIT)
        pos = {k: i for i, k in enumerate(perm)}
        nd, na = len(dve_fulls), len(act_fulls)
        acc_d = smallp.tile([128, nd + nsplit], f32)
        acc_a = smallp.tile([128, na + nsplit], f32)

        def _p1_dve(src_ap, col):
            s = scr.tile([128, maxQ], bf, tag="scr")
            nc.vector.tensor_scalar(
                out=s[:, 0:src_ap.shape[1]], in0=src_ap[:],
                scalar1=0.0, scalar2=0.0,
                op0=mybir.AluOpType.add, op1=mybir.AluOpType.add,
                accum_out=acc_d[:, col:col + 1])

        def _p1_act(src_ap, col):
            s = scr.tile([128, maxQ], bf, tag="scr2")
            nc.scalar.activation(
                out=s[:, 0:src_ap.shape[1]], in_=src_ap[:],
                func=mybir.ActivationFunctionType.Identity,
                accum_out=acc_a[:, col:col + 1])

        di = ai = 0
        for k in range(K):
            q = int(Qk[k])
            if k in _P1_SPLIT:
                half = (q // 2) // 8 * 8
                i = _P1_SPLIT.index(k)
                _p1_dve(feat_k[k][:, 0:half], nd + i)
                _p1_act(feat_k[k][:, half:q], na + i)
            elif k in _P1_ACT_SET:
                _p1_act(feat_k[k][:, 0:q], ai); ai += 1
            else:
                _p1_dve(feat_k[k][:, 0:q], di); di += 1

        mu_rep = psum1.tile([128, K], f32)
        nc.tensor.matmul(out=mu_rep[:, 0:nd], lhsT=fold_t[:],
                         rhs=acc_d[:, 0:nd],
                         start=True, stop=False, skip_group_check=True)
        nc.tensor.matmul(out=mu_rep[:, nd:nd + na], lhsT=fold_t[:],
                         rhs=acc_a[:, 0:na],
                         start=True, stop=False, skip_group_check=True)
        nc.tensor.matmul(out=mu_rep[:, K - nsplit:K], lhsT=fold_t[:],
                         rhs=acc_d[:, nd:nd + nsplit],
                         start=True, stop=False, skip_group_check=True)
        nc.tensor.matmul(out=mu_rep[:, K - nsplit:K], lhsT=fold_t[:],
                         rhs=acc_a[:, na:na + nsplit],
                         start=False, stop=True, skip_group_check=True)
        neg_mu = smallp.tile([128, K], f32)
        nc.vector.tensor_scalar(out=neg_mu[:], in0=mu_rep[:, 0:K],
                                scalar1=float(-1.0 / P), scalar2=None,
                                op0=mybir.AluOpType.mult)
        mu_col = smallp.tile([128, K], f32)
        nc.vector.tensor_scalar(out=mu_col[:], in0=neg_mu[:],
                                scalar1=-1.0, scalar2=None,
                                op0=mybir.AluOpType.mult)
        nc.sync.dma_start(out=mu_out[:], in_=mu_col[:])

        # ---- pass 2: per-cluster member distances on own half ----
        cs_a = psum1.tile([1, 288], f32, tag="cs_a")
        cs_b = psum1.tile([1, 288], f32, tag="cs_b")
        colbase = [0]
        for k in range(K):
            colbase.append(colbase[-1] + 4 * nch_k[k])
        half_cols = colbase[8]
        for k in range(K):
            hh = int(Hk[k])
            nch = nch_k[k]
            dsq = dsqp.tile([128, maxH], bf, tag="dsq")
            if k in _ACT_SQUARE_SET:
                # fused: Square(feat + (-mu)) in one ACT op
                nc.scalar.activation(
                    out=dsq[:, 0:hh], in_=feat_k[k][:, 0:hh],
                    func=mybir.ActivationFunctionType.Square,
                    bias=neg_mu[:, pos[k]:pos[k] + 1])
            else:
                diff = diffp.tile([128, maxH], bf, tag="diff")
                nc.vector.tensor_scalar(
                    out=diff[:, 0:hh], in0=feat_k[k][:, 0:hh],
                    scalar1=mu_col[:, pos[k]:pos[k] + 1], scalar2=None,
                    op0=mybir.AluOpType.subtract)
                sq_eng = nc.gpsimd if k in _POOL_SQUARE_SET else nc.vector
                sq_eng.tensor_tensor(
                    out=dsq[:, 0:hh], in0=diff[:, 0:hh], in1=diff[:, 0:hh],
                    op=mybir.AluOpType.mult)
            a2 = psum.tile([128, 4 * maxnch], f32, tag="a2")
            # zero the full window (partial chunks leave rows unwritten)
            nc.tensor.matmul(out=a2[:, 0:4 * nch], lhsT=feat_k[0][:, 0:128],
                             rhs=zeros_t[:, 0:4 * nch], start=True, stop=False)
            for ci in range(nch):
                cols = min(128, hh - 128 * ci)
                nc.tensor.matmul(
                    out=a2[0:cols, 4 * ci:4 * ci + 4],
                    lhsT=dsq[:, 128 * ci:128 * ci + cols],
                    rhs=sel4_t[:],
                    start=False, stop=(ci == nch - 1),
                    skip_group_check=True)
            dt = dtp.tile([128, 4 * maxnch], bf, tag="dt")
            nc.scalar.activation(
                out=dt[:, 0:4 * nch], in_=a2[:, 0:4 * nch],
                func=mybir.ActivationFunctionType.Sqrt)
            cs = cs_a if k < 8 else cs_b
            cb = colbase[k] - (0 if k < 8 else half_cols)
            nc.tensor.matmul(out=cs[:, cb:cb + 4 * nch],
                             lhsT=ones_b[:], rhs=dt[:, 0:4 * nch],
                             start=True, stop=True)
            if k == 7:
                s_sb = smallp.tile([1, scols], f32)
                nc.vector.tensor_scalar(
                    out=s_sb[:, 0:half_cols], in0=cs_a[:, 0:half_cols],
                    scalar1=1.0, scalar2=None, op0=mybir.AluOpType.mult)
                nc.sync.dma_start(out=s_out[:, 0:half_cols],
                                  in_=s_sb[:, 0:half_cols])

        nc.vector.tensor_scalar(out=s_sb[:, half_cols:scols],
                                in0=cs_b[:, 0:scols - half_cols], scalar1=1.0,
                                scalar2=None, op0=mybir.AluOpType.mult)
        nc.sync.dma_start(out=s_out[:, half_cols:scols],
                          in_=s_sb[:, half_cols:scols])

    nc.finalize()
    return nc


def _host_losses(mu, S_members, counts):
    """Assemble the four scalar losses in float64 given per-image mu
    (N,C,K), member distance sums S (N,K) and member counts (N,K)."""
    mu = mu.astype(np.float64)
    sq_mu = np.sum(mu * mu, axis=1)                       # (N,K)
    rt = np.sqrt(sq_mu)
    v = (S_members + (P - counts) * rt) / P               # (N,K)
    variance_loss = np.mean(np.mean(np.maximum(v - DELTA_V, 0.0), axis=1))

    gram = np.einsum('nck,ncj->nkj', mu, mu)
    sqd = sq_mu[:, :, None] + sq_mu[:, None, :] - 2.0 * gram
    eye = np.eye(K, dtype=bool)
    sqd = np.where(eye[None], 0.0, np.maximum(sqd, 0.0))
    D = np.sqrt(sqd)
    mean_dist = D.sum(axis=2) / (K - 1)
    distance_loss = np.sum(np.maximum(2.0 * DELTA_D - mean_dist, 0.0)) / (N * K)

    normalization_loss = np.mean(np.mean(rt, axis=1))
    total = ALPHA * variance_loss + BETA * distance_loss + GAMMA * normalization_loss
    return total, variance_loss, distance_loss, normalization_loss


def kernel(features, ground_truth):
    from concourse.bass_utils import run_bass_kernel_spmd

    feat = np.asarray(features, dtype=np.float32)
    gt = np.asarray(ground_truth)
    gt_flat = gt.reshape(N, P).astype(np.int64)

    idx, Hk, Qk, offs, F = _build_layout(gt_flat)
    bufs, npad = _build_core_inputs(feat, idx, Hk, Qk, offs, F)

    maxnch = max(int(math.ceil(Hk[k] / 128)) for k in range(K))
    sel4 = np.zeros((128, 4), dtype=_BF16)
    for j in range(NJ):
        sel4[j * 32:(j + 1) * 32, j] = 1
    zeros4 = np.zeros((128, 4 * maxnch), dtype=_BF16)
    fold = np.zeros((128, 128), dtype=np.float32)
    for j in range(NJ):
        for jp in range(NJ):
            fold[j * 32 + np.arange(32), jp * 32 + np.arange(32)] = 1
    ones1 = np.ones((128, 1), dtype=_BF16)

    nc = _build_program(Hk, Qk, offs, F)
    in_maps = [{"feat": bufs[c], "sel4": sel4, "zeros4": zeros4,
                "fold": fold, "ones1": ones1} for c in range(NCORES)]
    res = run_bass_kernel_spmd(nc, in_maps, list(range(NCORES)))
    results = res.results

    counts = np.stack([np.bincount(gt_flat[n], minlength=K) for n in range(N)])
    mu = np.zeros((N, C, K), dtype=np.float32)
    S_members = np.zeros((N, K), dtype=np.float64)
    for n in range(N):
        mu[n][:, _mu_perm()] = results[2 * n]["mu_out"][0:32, :]
    sq_mu_h = np.sum(mu.astype(np.float64) ** 2, axis=1)
    rt_h = np.sqrt(sq_mu_h)
    for core in range(NCORES):
        n = core // 2
        cols = results[core]["s_out"][0].astype(np.float64)
        cb = 0
        s_dev = np.zeros(K)
        for k in range(K):
            ncols = 4 * int(math.ceil(Hk[k] / 128))
            s_dev[k] = cols[cb:cb + ncols].sum()
            cb += ncols
        S_members[n] += s_dev - npad[core] * rt_h[n]

    total, var, dist, norm = _host_losses(mu, S_members, counts.astype(np.float64))
    return (np.float32(total), np.float32(var), np.float32(dist),
            np.float32(norm), mu)


if __name__ == "__main__":
    feat = np.load(os.path.join(os.path.dirname(__file__), "ref_inputs_feat.npy"))
    gt = np.load(os.path.join(os.path.dirname(__file__), "ref_inputs_gt.npy"))
    outs = kernel(feat, gt)
    for name, o in zip(["total", "var", "dist", "norm", "mu"], outs):
        print(name, np.asarray(o).ravel()[:4])


# revision 31
# speedup vs baseline: 1.0507x; 1.0507x over previous
"""Trainium2 Bass kernel for nn_Cluster_loss (discriminative/cluster loss).

Strategy (8 NeuronCores, full inputs in / full outputs back):
  - Host sorts each image's pixels by ground-truth cluster label into 16
    contiguous segments, split over 4 partition row-groups (j) of 32
    channels each -> feature layout (128, F) bf16 per core.
  - One full image per core (2 cores per image). Each core computes the
    image's cluster sums (mu) redundantly -- this avoids any mid-kernel
    cross-core collective.  Each core runs the per-pixel distance pass
    on the half of the pixels it "owns" (placed first in each segment).
  - Pass 1 (mu): per-segment DVE tensor_scalar with accum_out (free-axis
    accumulate), then a tiny fp32 matmul folds the 4 j-groups.
  - Pass 2: diff = feat - mu[cluster] via per-partition scalar subtract,
    square, then per-128-column-chunk matmul with the chunk as the
    stationary operand and a block-diagonal selector as moving operand:
    out[f, j] = sum_c diffsq[(j,c), f]  (pixels land on partitions).
    ACT Sqrt moves distances to SBUF; per-cluster ones-matmul column
    sums feed per-(chunk,j) partial sums shipped to the host.
  - Host assembles the scalar losses (float64) from mu + per-cluster
    distance sums, correcting analytically for zero pad pixels.
"""

import math
import os
import sys

import numpy as np

try:
    import ml_dtypes
except ImportError:  # pragma: no cover
    ml_dtypes = None

_TRN_REPO = "/opt/trn_rl_repo"
if _TRN_REPO not in sys.path:
    sys.path.insert(0, _TRN_REPO)

N, C, H, W = 4, 32, 256, 512
P = H * W          # pixels per image
K = 16             # clusters
NJ = 4             # partition row-groups (4 x 32 channels = 128 partitions)
NCORES = 8
DELTA_V = 0.2
DELTA_D = 0.2
ALPHA, BETA, GAMMA = 1.0, 1.0, 0.001

_BF16 = np.dtype(ml_dtypes.bfloat16) if ml_dtypes is not None else None

# Engine assignment knobs (load balance).
# pass 1 accumulation: which engine reduces each cluster segment.
# Interleaved with DMA arrival order so both engines chase the stream;
# ACT is faster per segment (1.95us vs 2.67us) so it takes more + the tail.
_P1_ACT_SET = frozenset({1, 3, 5, 7, 9, 11})
# last-arriving segments: accumulate the two halves on DVE+ACT in parallel
_P1_SPLIT = (12, 13, 14, 15)
# pass 2: ACT does fused Square(feat-mu) for these; DVE two-op for rest
_ACT_SQUARE_SET = frozenset({0, 2, 4, 6, 8, 10, 12})
# of the DVE-route clusters, these get their square on GpSimd (idle engine)
_POOL_SQUARE_SET = frozenset()


def _mu_perm():
    """Column order of mu_out (must match _build_program's perm)."""
    dve_fulls = [k for k in range(K)
                 if k not in _P1_ACT_SET and k not in _P1_SPLIT]
    return dve_fulls + sorted(_P1_ACT_SET) + list(_P1_SPLIT)


def _roundup(x, m):
    return (x + m - 1) // m * m


def _build_layout(gt_flat):
    """Compute global (shared across cores) segment geometry from labels.

    Returns dict with per-cluster H (own-half region width) and Q = 2H
    (full segment width), plus offsets.  All multiples of 8 columns.
    """
    # idx[n][k] = pixel indices of cluster k in image n
    idx = [[np.nonzero(gt_flat[n] == k)[0] for k in range(K)] for n in range(N)]
    Hk = np.zeros(K, dtype=np.int64)
    for k in range(K):
        m = 0
        for n in range(N):
            L = len(idx[n][k])
            for j in range(NJ):
                Lj = L // NJ + (1 if j < L % NJ else 0)
                m = max(m, (Lj + 1) // 2)
        Hk[k] = _roundup(max(m, 8), 8)
    Qk = 2 * Hk
    offs = np.zeros(K + 1, dtype=np.int64)
    np.cumsum(Qk, out=offs[1:])
    return idx, Hk, Qk, offs, int(offs[-1])


def _build_core_inputs(feat, idx, Hk, Qk, offs, F):
    """Per-core (128, F) bf16 feature buffers + pad bookkeeping.

    Core c handles image n = c // 2, half h = c % 2.  For each (j, k)
    the segment columns are [off, off+Hk): own pixels then zero pads;
    [off+Hk, off+Qk): partner pixels then zero pads.
    Returns (bufs, npad[core, k] total own-region pad count per cluster).
    """
    bufs = []
    npad = np.zeros((NCORES, K), dtype=np.int64)
    feat_bf = [np.ascontiguousarray(feat[n].reshape(C, P)).astype(_BF16)
               for n in range(N)]
    for core in range(NCORES):
        n, h = core // 2, core % 2
        colsrc = np.zeros((NJ, F), dtype=np.int64)
        valid = np.zeros((NJ, F), dtype=bool)
        for k in range(K):
            ids = idx[n][k]
            L = len(ids)
            pos = 0
            for j in range(NJ):
                Lj = L // NJ + (1 if j < L % NJ else 0)
                part = ids[pos:pos + Lj]
                pos += Lj
                a_sz = (Lj + 1) // 2
                own = part[:a_sz] if h == 0 else part[a_sz:]
                other = part[a_sz:] if h == 0 else part[:a_sz]
                o = offs[k]
                colsrc[j, o:o + len(own)] = own
                valid[j, o:o + len(own)] = True
                npad[core, k] += Hk[k] - len(own)
                o2 = offs[k] + Hk[k]
                colsrc[j, o2:o2 + len(other)] = other
                valid[j, o2:o2 + len(other)] = True
        buf = np.zeros((128, F), dtype=_BF16)
        fb = feat_bf[n]
        for j in range(NJ):
            block = fb[:, colsrc[j]]          # (32, F) gather
            block[:, ~valid[j]] = 0
            buf[j * 32:(j + 1) * 32, :] = block
        bufs.append(buf)
    return bufs, npad


def _build_program(Hk, Qk, offs, F):
    import concourse.bass as bass
    import concourse.tile as tile
    from concourse import bacc, mybir
    from contextlib import ExitStack

    bf = mybir.dt.bfloat16
    f32 = mybir.dt.float32
    nc = bacc.Bacc("TRN2", target_bir_lowering=False, debug=False)

    nch_k = [int(math.ceil(Hk[k] / 128)) for k in range(K)]
    maxnch = max(nch_k)

    feat_in = nc.declare_dram_parameter("feat", [128, F], bf, isOutput=False)
    sel4_in = nc.declare_dram_parameter("sel4", [128, 4], bf, isOutput=False)
    zeros_in = nc.declare_dram_parameter("zeros4", [128, 4 * maxnch], bf,
                                         isOutput=False)
    fold_in = nc.declare_dram_parameter("fold", [128, 128], f32, isOutput=False)
    ones_in = nc.declare_dram_parameter("ones1", [128, 1], bf, isOutput=False)
    mu_out = nc.declare_dram_parameter("mu_out", [128, K], f32, isOutput=True)
    scols = 4 * sum(nch_k)
    s_out = nc.declare_dram_parameter("s_out", [1, scols], f32, isOutput=True)

    maxQ = int(max(Qk))
    maxH = int(max(Hk))

    with tile.TileContext(nc) as tc, ExitStack() as ctx:
        consts = ctx.enter_context(tc.tile_pool(name="consts", bufs=1))
        featp = ctx.enter_context(tc.tile_pool(name="featp", bufs=1))
        scr = ctx.enter_context(tc.tile_pool(name="scr", bufs=2))
        diffp = ctx.enter_context(tc.tile_pool(name="diffp", bufs=4))
        dsqp = ctx.enter_context(tc.tile_pool(name="dsqp", bufs=4))
        dtp = ctx.enter_context(tc.tile_pool(name="dtp", bufs=4))
        smallp = ctx.enter_context(tc.tile_pool(name="smallp", bufs=1))
        psum = ctx.enter_context(tc.tile_pool(name="psum", bufs=5, space="PSUM"))
        psum1 = ctx.enter_context(tc.tile_pool(name="psum1", bufs=1, space="PSUM"))

        # warm the ACT Sqrt table before anything else needs ACT
        warm = smallp.tile([128, 8], bf)
        nc.vector.memset(warm[:], 0.0)
        nc.scalar.activation(out=warm[:], in_=warm[:],
                             func=mybir.ActivationFunctionType.Sqrt)

        sel4_t = consts.tile([128, 4], bf)
        nc.scalar.dma_start(out=sel4_t[:], in_=sel4_in[:])
        zeros_t = consts.tile([128, 4 * maxnch], bf)
        nc.scalar.dma_start(out=zeros_t[:], in_=zeros_in[:])
        fold_t = consts.tile([128, 128], f32)
        nc.scalar.dma_start(out=fold_t[:], in_=fold_in[:])
        ones_b = consts.tile([128, 1], bf)
        nc.scalar.dma_start(out=ones_b[:], in_=ones_in[:])

        # feature loads: split the issue across two otherwise-idle
        # sequencers (SP + GpSimd) -- one sequencer can only start a DMA
        # every ~1.2us, which otherwise rate-limits the whole load phase.
        feat_k = []
        for k in range(K):
            o, q = int(offs[k]), int(Qk[k])
            ft = featp.tile([128, q], bf, tag=f"feat{k}")
            eng = nc.sync if k % 2 == 0 else nc.gpsimd
            if k in _P1_SPLIT:
                half = (q // 2) // 8 * 8
                eng.dma_start(out=ft[:, 0:half], in_=feat_in[:, o:o + half])
                eng.dma_start(out=ft[:, half:q], in_=feat_in[:, o + half:o + q])
            else:
                eng.dma_start(out=ft[:], in_=feat_in[:, o:o + q])
            feat_k.append(ft)

        # ---- pass 1: per-segment accumulate ----
        # Separate per-engine accumulator tiles: a shared tile would
        # serialize DVE/ACT accums on whole-tile WAW tracking.
        nsplit = len(_P1_SPLIT)
        dve_fulls = [k for k in range(K)
                     if k not in _P1_ACT_SET and k not in _P1_SPLIT]
        act_fulls = sorted(_P1_ACT_SET)
        # mu_rep column order: [dve_fulls | act_fulls | splits]
        perm = dve_fulls + act_fulls + list(_P1_SPLIT)
        pos = {k: i for i, k in enumerate(perm)}
        nd, na = len(dve_fulls), len(act_fulls)
        acc_d = smallp.tile([128, nd + nsplit], f32)
        acc_a = smallp.tile([128, na + nsplit], f32)

        def _p1_dve(src_ap, col):
            s = scr.tile([128, maxQ], bf, tag="scr")
            nc.vector.tensor_scalar(
                out=s[:, 0:src_ap.shape[1]], in0=src_ap[:],
                scalar1=0.0, scalar2=0.0,
                op0=mybir.AluOpType.add, op1=mybir.AluOpType.add,
                accum_out=acc_d[:, col:col + 1])

        def _p1_act(src_ap, col):
            s = scr.tile([128, maxQ], bf, tag="scr2")
            nc.scalar.activation(
                out=s[:, 0:src_ap.shape[1]], in_=src_ap[:],
                func=mybir.ActivationFunctionType.Identity,
                accum_out=acc_a[:, col:col + 1])

        di = ai = 0
        for k in range(K):
            q = int(Qk[k])
            if k in _P1_SPLIT:
                half = (q // 2) // 8 * 8
                i = _P1_SPLIT.index(k)
                _p1_dve(feat_k[k][:, 0:half], nd + i)
                _p1_act(feat_k[k][:, half:q], na + i)
            elif k in _P1_ACT_SET:
                _p1_act(feat_k[k][:, 0:q], ai); ai += 1
            else:
                _p1_dve(feat_k[k][:, 0:q], di); di += 1

        mu_rep = psum1.tile([128, K], f32)
        nc.tensor.matmul(out=mu_rep[:, 0:nd], lhsT=fold_t[:],
                         rhs=acc_d[:, 0:nd],
                         start=True, stop=False, skip_group_check=True)
        nc.tensor.matmul(out=mu_rep[:, nd:nd + na], lhsT=fold_t[:],
                         rhs=acc_a[:, 0:na],
                         start=True, stop=False, skip_group_check=True)
        nc.tensor.matmul(out=mu_rep[:, K - nsplit:K], lhsT=fold_t[:],
                         rhs=acc_d[:, nd:nd + nsplit],
                         start=True, stop=False, skip_group_check=True)
        nc.tensor.matmul(out=mu_rep[:, K - nsplit:K], lhsT=fold_t[:],
                         rhs=acc_a[:, na:na + nsplit],
                         start=False, stop=True, skip_group_check=True)
        neg_mu = smallp.tile([128, K], f32)
        nc.vector.tensor_scalar(out=neg_mu[:], in0=mu_rep[:, 0:K],
                                scalar1=float(-1.0 / P), scalar2=None,
                                op0=mybir.AluOpType.mult)
        mu_col = smallp.tile([128, K], f32)
        nc.vector.tensor_scalar(out=mu_col[:], in0=neg_mu[:],
                                scalar1=-1.0, scalar2=None,
                                op0=mybir.AluOpType.mult)
        nc.sync.dma_start(out=mu_out[:], in_=mu_col[:])

        # ---- pass 2: per-cluster member distances on own half ----
        cs_a = psum1.tile([1, 288], f32, tag="cs_a")
        cs_b = psum1.tile([1, 288], f32, tag="cs_b")
        colbase = [0]
        for k in range(K):
            colbase.append(colbase[-1] + 4 * nch_k[k])
        half_cols = colbase[8]
        for k in range(K):
            hh = int(Hk[k])
            nch = nch_k[k]
            dsq = dsqp.tile([128, maxH], bf, tag="dsq")
            if k in _ACT_SQUARE_SET:
                # fused: Square(feat + (-mu)) in one ACT op
                nc.scalar.activation(
                    out=dsq[:, 0:hh], in_=feat_k[k][:, 0:hh],
                    func=mybir.ActivationFunctionType.Square,
                    bias=neg_mu[:, pos[k]:pos[k] + 1])
            else:
                diff = diffp.tile([128, maxH], bf, tag="diff")
                nc.vector.tensor_scalar(
                    out=diff[:, 0:hh], in0=feat_k[k][:, 0:hh],
                    scalar1=mu_col[:, pos[k]:pos[k] + 1], scalar2=None,
                    op0=mybir.AluOpType.subtract)
                sq_eng = nc.gpsimd if k in _POOL_SQUARE_SET else nc.vector
                sq_eng.tensor_tensor(
                    out=dsq[:, 0:hh], in0=diff[:, 0:hh], in1=diff[:, 0:hh],
                    op=mybir.AluOpType.mult)
            a2 = psum.tile([128, 4 * maxnch], f32, tag="a2")
            # zero the full window (partial chunks leave rows unwritten)
            nc.tensor.matmul(out=a2[:, 0:4 * nch], lhsT=feat_k[0][:, 0:128],
                             rhs=zeros_t[:, 0:4 * nch], start=True, stop=False)
            for ci in range(nch):
                cols = min(128, hh - 128 * ci)
                nc.tensor.matmul(
                    out=a2[0:cols, 4 * ci:4 * ci + 4],
                    lhsT=dsq[:, 128 * ci:128 * ci + cols],
                    rhs=sel4_t[:],
                    start=False, stop=(ci == nch - 1),
                    skip_group_check=True)
            dt = dtp.tile([128, 4 * maxnch], bf, tag="dt")
            nc.scalar.activation(
                out=dt[:, 0:4 * nch], in_=a2[:, 0:4 * nch],
                func=mybir.ActivationFunctionType.Sqrt)
            cs = cs_a if k < 8 else cs_b
            cb = colbase[k] - (0 if k < 8 else half_cols)
            nc.tensor.matmul(out=cs[:, cb:cb + 4 * nch],
                             lhsT=ones_b[:], rhs=dt[:, 0:4 * nch],
                             start=True, stop=True)
            if k == 7:
                s_sb = smallp.tile([1, scols], f32)
                nc.vector.tensor_scalar(
                    out=s_sb[:, 0:half_cols], in0=cs_a[:, 0:half_cols],
                    scalar1=1.0, scalar2=None, op0=mybir.AluOpType.mult)
                nc.sync.dma_start(out=s_out[:, 0:half_cols],
                                  in_=s_sb[:, 0:half_cols])

        nc.vector.tensor_scalar(out=s_sb[:, half_cols:scols],
                                in0=cs_b[:, 0:scols - half_cols], scalar1=1.0,
                                scalar2=None, op0=mybir.AluOpType.mult)
        nc.sync.dma_start(out=s_out[:, half_cols:scols],
                          in_=s_sb[:, half_cols:scols])

    nc.finalize()
    return nc


def _host_losses(mu, S_members, counts):
    """Assemble the four scalar losses in float64 given per-image mu
    (N,C,K), member distance sums S (N,K) and member counts (N,K)."""
    mu = mu.astype(np.float64)
    sq_mu = np.sum(mu * mu, axis=1)                       # (N,K)
    rt = np.sqrt(sq_mu)
    v = (S_members + (P - counts) * rt) / P               # (N,K)
    variance_loss = np.mean(np.mean(np.maximum(v - DELTA_V, 0.0), axis=1))

    gram = np.einsum('nck,ncj->nkj', mu, mu)
    sqd = sq_mu[:, :, None] + sq_mu[:, None, :] - 2.0 * gram
    eye = np.eye(K, dtype=bool)
    sqd = np.where(eye[None], 0.0, np.maximum(sqd, 0.0))
    D = np.sqrt(sqd)
    mean_dist = D.sum(axis=2) / (K - 1)
    distance_loss = np.sum(np.maximum(2.0 * DELTA_D - mean_dist, 0.0)) / (N * K)

    normalization_loss = np.mean(np.mean(rt, axis=1))
    total = ALPHA * variance_loss + BETA * distance_loss + GAMMA * normalization_loss
    return total, variance_loss, distance_loss, normalization_loss


def kernel(features, ground_truth):
    from concourse.bass_utils import run_bass_kernel_spmd

    feat = np.asarray(features, dtype=np.float32)
    gt = np.asarray(ground_truth)
    gt_flat = gt.reshape(N, P).astype(np.int64)

    idx, Hk, Qk, offs, F = _build_layout(gt_flat)
    bufs, npad = _build_core_inputs(feat, idx, Hk, Qk, offs, F)

    maxnch = max(int(math.ceil(Hk[k] / 128)) for k in range(K))
    sel4 = np.zeros((128, 4), dtype=_BF16)
    for j in range(NJ):
        sel4[j * 32:(j + 1) * 32, j] = 1
    zeros4 = np.zeros((128, 4 * maxnch), dtype=_BF16)
    fold = np.zeros((128, 128), dtype=np.float32)
    for j in range(NJ):
        for jp in range(NJ):
            fold[j * 32 + np.arange(32), jp * 32 + np.arange(32)] = 1
    ones1 = np.ones((128, 1), dtype=_BF16)

    nc = _build_program(Hk, Qk, offs, F)
    in_maps = [{"feat": bufs[c], "sel4": sel4, "zeros4": zeros4,
                "fold": fold, "ones1": ones1} for c in range(NCORES)]
    res = run_bass_kernel_spmd(nc, in_maps, list(range(NCORES)))
    results = res.results

    counts = np.stack([np.bincount(gt_flat[n], minlength=K) for n in range(N)])
    mu = np.zeros((N, C, K), dtype=np.float32)
    S_members = np.zeros((N, K), dtype=np.float64)
    for n in range(N):
        mu[n][:, _mu_perm()] = results[2 * n]["mu_out"][0:32, :]
    sq_mu_h = np.sum(mu.astype(np.float64) ** 2, axis=1)
    rt_h = np.sqrt(sq_mu_h)
    for core in range(NCORES):
        n = core // 2
        cols = results[core]["s_out"][0].astype(np.float64)
        cb = 0
        s_dev = np.zeros(K)
        for k in range(K):
            ncols = 4 * int(math.ceil(Hk[k] / 128))
            s_dev[k] = cols[cb:cb + ncols].sum()
            cb += ncols
        S_members[n] += s_dev - npad[core] * rt_h[n]

    total, var, dist, norm = _host_losses(mu, S_members, counts.astype(np.float64))
    return (np.float32(total), np.float32(var), np.float32(dist),
            np.float32(norm), mu)


if __name__ == "__main__":
    feat = np.load(os.path.join(os.path.dirname(__file__), "ref_inputs_feat.npy"))
    gt = np.load(os.path.join(os.path.dirname(__file__), "ref_inputs_gt.npy"))
    outs = kernel(feat, gt)
    for name, o in zip(["total", "var", "dist", "norm", "mu"], outs):
        print(name, np.asarray(o).ravel()[:4])
